# Initial kernel scaffold
#
"""Bi-LSTM Trainium2 kernel: B=64, T=256, D=512, H=512, fp32 I/O.

Sharding: 8 cores = 4 batch groups x 2 directions. Each core runs the full
time recurrence for its 16-sample shard in one direction (the backward
direction is handled by feeding that core a time-reversed input and
un-reversing its output on the host).

On-device layout is fully transposed: hidden/gate dims on SBUF partitions,
batch on the free dim. The recurrent matmul keeps the weight tile as the
stationary (lhsT) operand so the gate output lands transposed in PSUM,
which makes every elementwise op a [128, 64]-shaped op and removes any
per-step transposes.

Phase 1 precomputes gx[t] = x_t @ Wx + b for all t into DRAM scratch;
phase 2 runs the sequential recurrence g = gx[t] + h @ Wh plus the LSTM
cell elementwise. In the bf16 variant, gx is injected into PSUM with an
identity matmul (so the gate nonlinearities read PSUM directly, no DVE
adds), and phase-1 work is interleaved between recurrence steps so it
executes inside the PE gaps left by the elementwise tail.
"""

import sys

for _p in ("/opt/trn_rl_repo",):
    if _p not in sys.path:
        sys.path.append(_p)

import numpy as np
import ml_dtypes

import concourse.bass as bass
import concourse.mybir as mybir
from concourse import tile
from concourse.bass_utils import run_bass_kernel_spmd
from concourse.masks import make_identity

B, T, D, H = 64, 256, 512, 512
NCORES = 8
GROUPS = 4
BS = B // GROUPS          # batch rows per core
NK = H // 128             # contraction tiles over the hidden dim
NM = (4 * H) // 128       # output tiles over the gate dim
BLK_T = 32                # timesteps per phase-1 block
F32 = mybir.dt.float32
F32R = mybir.dt.float32r
BF16 = mybir.dt.bfloat16

# process gate blocks in [i, f, c, o] order so the output-gate chain is the
# only elementwise work left after the last matmul of a step
M_ORDER = list(range(0, 8)) + list(range(12, 16)) + list(range(8, 12))


def _patch_tail_drain():
    """This image's walrus rejects more than one sync-wait per engine
    instruction (and any wait on a self-loading 4-byte matmul). Tile
    attaches one wait per outstanding semaphore, so split the excess onto
    nofuse nops committed just before the instruction they guard (same
    engine -> identical semantics)."""
    import bass_rust
    from concourse.vector_clock import ScopedClock

    if getattr(tile.TileContext, "_drain_split_patched", False):
        return

    def _drain_and_barrier(self, tick_clock, wait_clock):
        drain_inst = self.nc.sync.drain()
        wait_clock.add_sem_waits(
            drain_inst.ins, ScopedClock({None: tick_clock.global_clock})
        )
        si = drain_inst.ins.sync_info
        if si is not None and len(si.on_wait) > 1:
            waits = list(si.on_wait)
            drain_inst.ins.sync_info = bass_rust.SyncInfo(
                on_wait=waits[:1], on_update=list(si.on_update)
            )
            for i in range(1, len(waits)):
                nop = self.nc.sync.nop(nofuse=True)
                nop.ins.sync_info = bass_rust.SyncInfo(
                    on_wait=waits[i : i + 1], on_update=[]
                )

        self.nc.all_engine_barrier()
        assert self.sems is not None
        popped = self.nc._tile_sem_poison_stack.pop()
        assert popped is self._sem_poison
        self.nc.clear_and_free_semaphores(list(self.sems.allocated().values()))
        self.nc.all_engine_barrier()

    tile.TileContext._drain_and_barrier = _drain_and_barrier

    orig_commit = tile.TileContext._commit_instruction

    def _commit_instruction(self, inst, lazy_reg_writes: bool = True):
        si = getattr(inst, "sync_info", None)
        limit = 0 if isinstance(inst, mybir.InstMatmult) else 1
        if (
            si is not None
            and len(si.on_wait) > limit
            and inst.engine != mybir.EngineType.Unassigned
        ):
            waits = list(si.on_wait)
            keep = waits[len(waits) - limit :] if limit else []
            for w in waits[: len(waits) - limit]:
                nop = mybir.InstNoOp(
                    name=f"I-{self.nc.next_id()}",
                    sync_info=mybir.SyncInfo(on_wait=[w], on_update=[]),
                    bass_nofuse=True,
                    engine=inst.engine,
                )
                orig_commit(self, nop, lazy_reg_writes=False)
            inst.sync_info = mybir.SyncInfo(
                on_wait=keep, on_update=list(si.on_update)
            )
        return orig_commit(self, inst, lazy_reg_writes)

    tile.TileContext._commit_instruction = _commit_instruction
    tile.TileContext._drain_split_patched = True


def build(recur_dt: str = "bf16", with_mask: bool = False, t_steps: int = T,
          gx_out: bool = False, p1_dt: str | None = None):
    """Emit the per-core SPMD module.

    recur_dt: dtype of Wh/h/gx for the recurrence ('bf16' or 'f32').
    p1_dt: matmul dtype of the x-projection ('f32r' or 'f32').
    """
    _patch_tail_drain()
    bf = recur_dt == "bf16"
    DT = BF16 if bf else F32
    GXDT = BF16 if bf else F32
    if p1_dt is None:
        p1_dt = "f32r" if bf else "f32"
    P1DT = F32R if p1_dt == "f32r" else F32
    blk_t = min(BLK_T, t_steps)
    nblk = t_steps // blk_t

    nc = bass.Bass("TRN2", target_bir_lowering=False, debug=False,
                   num_devices=NCORES)

    xT = nc.dram_tensor("xT", [D, t_steps * BS], P1DT, kind="ExternalInput")
    wx = nc.dram_tensor("wx", [D, 4 * H], P1DT, kind="ExternalInput")
    wh = nc.dram_tensor("wh", [H, 4 * H], DT, kind="ExternalInput")
    bt = nc.dram_tensor("bt", [128, NM], F32, kind="ExternalInput")
    msk = (
        nc.dram_tensor("msk", [t_steps, 128, NK * BS], F32,
                       kind="ExternalInput")
        if with_mask
        else None
    )
    hout = nc.dram_tensor("hout", [t_steps, 128, NK * BS], DT,
                          kind="ExternalOutput")
    ngx = 2 if bf else 1  # bf16 gx is stored as hi+lo bf16 pair
    gxd = nc.dram_tensor("gx_scratch", [128, t_steps, ngx * NM * BS], GXDT,
                         kind="ExternalOutput" if gx_out else "Internal")

    Act = mybir.ActivationFunctionType
    Alu = mybir.AluOpType

    with tile.TileContext(nc) as tc:
        with (
            tc.tile_pool(name="weights", bufs=1) as wpool,
            tc.tile_pool(name="state", bufs=1) as spool,
            tc.tile_pool(name="p1x", bufs=2) as xpool,
            tc.tile_pool(name="p1stg", bufs=2) as stgpool,
            tc.tile_pool(name="p1psum", bufs=2, space="PSUM") as p1ps,
            tc.tile_pool(name="p2psum", bufs=2, space="PSUM") as p2ps,
            tc.tile_pool(name="p2gx", bufs=4) as gxpool,
            tc.tile_pool(name="p2ew", bufs=2) as ewpool,
        ):
            wxs = wpool.tile([128, NK, 4 * H], P1DT)
            nc.gpsimd.dma_start(wxs[:], wx.ap().rearrange("(k p) n -> p k n", p=128))
            whs = wpool.tile([128, NK, 4 * H], DT)
            nc.gpsimd.dma_start(whs[:], wh.ap().rearrange("(k p) n -> p k n", p=128))
            bts = wpool.tile([128, NM], F32)
            nc.gpsimd.dma_start(bts[:], bt.ap())
            if bf:
                ident = wpool.tile([128, 128], BF16)
                make_identity(nc, ident[:])

            hT = spool.tile([128, NK, BS], DT)
            cT = spool.tile([128, NK, BS], F32)
            nc.vector.memset(hT[:], 0.0)
            nc.vector.memset(cT[:], 0.0)

            # ---- phase 1 machinery: gx[t] = x_t @ Wx + b ----
            xview = xT.ap().rearrange("(k p) n -> p k n", p=128)
            nfree = blk_t * BS
            p1_tiles: dict = {}
            anchor = [None]

            def p1_group(blk, m):
                """One m-tile of one phase-1 block: 4 matmuls + staging."""
                if m == 0:
                    xblk = xpool.tile([128, NK, nfree], P1DT, tag="xblk")
                    nc.gpsimd.dma_start(
                        xblk[:], xview[:, :, blk * nfree : (blk + 1) * nfree]
                    )
                    stg = stgpool.tile([128, blk_t, ngx, NM, BS], GXDT,
                                       tag="stg")
                    p1_tiles[blk] = (xblk, stg)
                xblk, stg = p1_tiles[blk]
                ps = p1ps.tile([128, nfree], F32, tag="p1ps")
                for k in range(NK):
                    mm = nc.tensor.matmul(
                        ps[:],
                        wxs[:, k, m * 128 : (m + 1) * 128],
                        xblk[:, k, :],
                        start=(k == 0),
                        stop=(k == NK - 1),
                    )
                    if k == 0 and anchor[0] is not None:
                        from concourse.bass import _add_dep_helper
                        _add_dep_helper(
                            mm.ins, anchor[0],
                            reason="pin p1 group behind its step",
                        )
                psv = ps[:].rearrange("p (t b) -> p t b", b=BS)
                if not bf:
                    if m % 2 == 0:
                        nc.vector.tensor_scalar(
                            stg[:, :, 0, m, :], psv, bts[:, m : m + 1], None,
                            Alu.add
                        )
                    else:
                        nc.scalar.activation(
                            stg[:, :, 0, m, :], psv, Act.Identity,
                            bias=bts[:, m : m + 1],
                        )
                else:
                    # hi = bf16(ps + b) on ACT; lo = bf16((ps + b) - hi) on DVE
                    nc.scalar.activation(
                        stg[:, :, 0, m, :], psv, Act.Identity,
                        bias=bts[:, m : m + 1],
                    )
                    nc.vector.scalar_tensor_tensor(
                        stg[:, :, 1, m, :], psv, bts[:, m : m + 1],
                        stg[:, :, 0, m, :], Alu.add, Alu.subtract,
                    )
                if m == NM - 1:
                    # split the store so early gx loads unblock sooner
                    qt = blk_t // 4 if blk_t % 4 == 0 else blk_t
                    for q in range(blk_t // qt):
                        nc.gpsimd.dma_start(
                            gxd.ap()[
                                :,
                                blk * blk_t + q * qt : blk * blk_t + (q + 1) * qt,
                                :,
                            ],
                            stg[:, q * qt : (q + 1) * qt].rearrange(
                                "p t g m b -> p t (g m b)"
                            ),
                        )
                    del p1_tiles[blk]

            # Interleave schedule: blocks 0/1 up front, block b's 16 groups
            # spread over steps [(b-2)*blk_t, (b-1)*blk_t) at 1 per 2 steps.
            sched: dict[int, list] = {}
            n_prologue = min(1, nblk)
            if bf:
                for b in range(n_prologue, nblk):
                    if b == 1:
                        slots = [2 * m for m in range(NM)]
                    elif b == 2:
                        slots = [16 + 3 * m for m in range(NM)]
                    else:
                        slots = [(b - 2) * blk_t + 3 * m for m in range(NM)]
                    for m, s in enumerate(slots):
                        sched.setdefault(s, []).append((b, m))
            else:
                n_prologue = nblk
            for b in range(n_prologue):
                for m in range(NM):
                    p1_group(b, m)

            # ---- phase 2: the recurrence ----
            for t in range(t_steps):
                gx = gxpool.tile([128, ngx, NM, BS], GXDT)
                nc.sync.dma_start(
                    gx[:],
                    gxd.ap()[:, t, :].rearrange("p (g m b) -> p g m b",
                                                b=BS, m=NM),
                )
                ps_if = p2ps.tile([128, 8, BS], F32, tag="ps_if")
                ps_c = p2ps.tile([128, 4, BS], F32, tag="ps_c")
                ps_o = p2ps.tile([128, 4, BS], F32, tag="ps_o")
                for m in M_ORDER:
                    if m < 8:
                        dst = ps_if[:, m, :]
                    elif m >= 12:
                        dst = ps_c[:, m - 12, :]
                    else:
                        dst = ps_o[:, m - 8, :]
                    if bf:
                        nc.tensor.matmul(dst, ident[:], gx[:, 0, m, :],
                                         start=True, stop=False)
                        nc.tensor.matmul(dst, ident[:], gx[:, 1, m, :],
                                         start=False, stop=False)
                    for k in range(NK):
                        mm = nc.tensor.matmul(
                            dst,
                            whs[:, k, m * 128 : (m + 1) * 128],
                            hT[:, k, :],
                            start=(not bf) and k == 0,
                            stop=(k == NK - 1),
                        )
                        anchor[0] = mm.ins
                    if m == 7:
                        sif = ewpool.tile([128, 8, BS], F32, tag="sif")
                        if bf:
                            nc.scalar.activation(sif[:], ps_if[:], Act.Sigmoid)
                        else:
                            nc.vector.tensor_tensor(
                                sif[:], ps_if[:], gx[:, 0, 0:8, :], Alu.add
                            )
                            nc.scalar.activation(sif[:], sif[:], Act.Sigmoid)
                    elif m == 15:
                        tcg = ewpool.tile([128, NK, BS], F32, tag="tcg")
                        if bf:
                            nc.scalar.activation(tcg[:], ps_c[:], Act.Tanh)
                        else:
                            nc.vector.tensor_tensor(
                                tcg[:], ps_c[:], gx[:, 0, 12:16, :], Alu.add
                            )
                            nc.scalar.activation(tcg[:], tcg[:], Act.Tanh)
                        t1 = ewpool.tile([128, NK, BS], F32, tag="t1")
                        nc.vector.tensor_tensor(
                            t1[:], sif[:, 4:8, :], cT[:], Alu.mult
                        )
                        t2 = ewpool.tile([128, NK, BS], F32, tag="t2")
                        nc.vector.tensor_tensor(
                            t2[:], sif[:, 0:4, :], tcg[:], Alu.mult
                        )
                        if with_mask:
                            cn = ewpool.tile([128, NK, BS], F32, tag="cn")
                            nc.vector.tensor_tensor(cn[:], t1[:], t2[:], Alu.add)
                            mt = ewpool.tile([128, NK * BS], F32, tag="mt")
                            nc.gpsimd.dma_start(mt[:], msk[t])
                            mtv = mt[:].rearrange("p (k b) -> p k b", b=BS)
                            cd = ewpool.tile([128, NK, BS], F32, tag="cd")
                            nc.vector.tensor_tensor(cd[:], cn[:], cT[:],
                                                    Alu.subtract)
                            nc.vector.tensor_tensor(cd[:], cd[:], mtv, Alu.mult)
                            nc.vector.tensor_tensor(cT[:], cT[:], cd[:], Alu.add)
                        else:
                            nc.vector.tensor_tensor(cT[:], t1[:], t2[:], Alu.add)
                        thc = ewpool.tile([128, NK, BS], F32, tag="thc")
                        nc.scalar.activation(thc[:], cT[:], Act.Tanh)
                # output gate chain + h update
                og = ewpool.tile([128, NK, BS], F32, tag="og")
                if bf:
                    nc.scalar.activation(og[:], ps_o[:], Act.Sigmoid)
                else:
                    nc.vector.tensor_tensor(og[:], ps_o[:], gx[:, 0, 8:12, :],
                                            Alu.add)
                    nc.scalar.activation(og[:], og[:], Act.Sigmoid)
                # threshold(o, 0.4): o if o > 0.4 else 0
                nc.vector.scalar_tensor_tensor(
                    og[:], og[:], 0.4, og[:], Alu.is_gt, Alu.mult
                )
                if with_mask:
                    hn = ewpool.tile([128, NK, BS], F32, tag="hn")
                    nc.vector.tensor_tensor(hn[:], og[:], thc[:], Alu.mult)
                    hd = ewpool.tile([128, NK, BS], F32, tag="hd")
                    nc.vector.tensor_tensor(hd[:], hn[:], hT[:], Alu.subtract)
                    nc.vector.tensor_tensor(hd[:], hd[:], mtv, Alu.mult)
                    nc.vector.tensor_tensor(hT[:], hT[:], hd[:], Alu.add)
                else:
                    nc.vector.tensor_tensor(hT[:], og[:], thc[:], Alu.mult)
                nc.sync.dma_start(hout[t], hT[:].rearrange("p k b -> p (k b)"))

                for b, m in sched.get(t, ()):
                    p1_group(b, m)
    return nc


_BUILD_CACHE: dict = {}


def _get_module(recur_dt: str, with_mask: bool, t_steps: int = T,
                p1_dt: str | None = None):
    key = (recur_dt, with_mask, t_steps, p1_dt)
    if key not in _BUILD_CACHE:
        _BUILD_CACHE[key] = build(recur_dt, with_mask, t_steps, p1_dt=p1_dt)
    return _BUILD_CACHE[key]


def _make_in_maps(x, mask, Wf, bf, Wb, bb, recur_dt: str, with_mask: bool,
                  t_steps: int = T):
    DTnp = ml_dtypes.bfloat16 if recur_dt == "bf16" else np.float32
    ws = {}
    for d, (W, bias) in enumerate(((Wf, bf), (Wb, bb))):
        W = np.asarray(W, np.float32)
        ws[d] = (
            np.ascontiguousarray(W[H:]),                 # wx (x rows), f32
            np.ascontiguousarray(W[:H].astype(DTnp)),    # wh (h rows)
            np.ascontiguousarray(
                np.asarray(bias, np.float32).reshape(NM, 128).T
            ),
        )
    in_maps = []
    for core in range(NCORES):
        g, d = core // 2, core % 2
        xs = np.asarray(x[g * BS : (g + 1) * BS, :t_steps], np.float32)
        ms = np.asarray(mask[g * BS : (g + 1) * BS, :t_steps], np.float32)
        if d == 1:
            xs = xs[:, ::-1]
            ms = ms[:, ::-1]
        # xT[dd, t*BS + b] = xs[b, t, dd]
        xTv = np.ascontiguousarray(
            xs.transpose(2, 1, 0).reshape(D, t_steps * BS)
        )
        wxv, whv, btv = ws[d]
        m = {"xT": xTv, "wx": wxv, "wh": whv, "bt": btv}
        if with_mask:
            m["msk"] = np.ascontiguousarray(
                np.broadcast_to(
                    ms.T[:, None, None, :], (t_steps, 128, NK, BS)
                ).reshape(t_steps, 128, NK * BS)
            )
        in_maps.append(m)
    return in_maps


def _assemble(results, t_steps: int = T):
    out = np.empty((B, t_steps, 2 * H), np.float32)
    for core in range(NCORES):
        g, d = core // 2, core % 2
        h = np.asarray(results[core]["hout"], np.float32)  # [t, 128, NK*BS]
        h = h.reshape(t_steps, 128, NK, BS).transpose(3, 0, 2, 1)  # [b,t,k,p]
        h = h.reshape(BS, t_steps, H)
        if d == 1:
            h = h[:, ::-1]
        out[g * BS : (g + 1) * BS, :, d * H : (d + 1) * H] = h
    return out


def run(x, mask, Wf, bf, Wb, bb, recur_dt="bf16", trace=False, t_steps: int = T,
        p1_dt: str | None = None, **spmd_kwargs):
    with_mask = not bool(np.all(np.asarray(mask) == 1.0))
    nc = _get_module(recur_dt, with_mask, t_steps, p1_dt)
    in_maps = _make_in_maps(x, mask, Wf, bf, Wb, bb, recur_dt, with_mask,
                            t_steps)
    res = run_bass_kernel_spmd(
        nc, in_maps, list(range(NCORES)), trace=trace, **spmd_kwargs
    )
    return _assemble(res.results, t_steps), res


def kernel(x, mask, Wf, bf, Wb, bb):
    out, _ = run(x, mask, Wf, bf, Wb, bb)
    return out



# revision 31
# speedup vs baseline: 1.2453x; 1.2453x over previous
"""Bi-LSTM Trainium2 kernel: B=64, T=256, D=512, H=512, fp32 I/O.

Sharding: 8 cores = 4 batch groups x 2 directions. Each core runs the full
time recurrence for its 16-sample shard in one direction (the backward
direction is handled by feeding that core a time-reversed input and
un-reversing its output on the host).

On-device layout is fully transposed: hidden/gate dims on SBUF partitions,
batch on the free dim. The recurrent matmul keeps the weight tile as the
stationary (lhsT) operand so the gate output lands transposed in PSUM,
which makes every elementwise op a [128, BS]-shaped op and removes any
per-step transposes.

Phase 1 precomputes gx[t] = x_t @ Wx + b for all t into DRAM scratch;
phase 2 runs the sequential recurrence g = gx[t] + h @ Wh plus the LSTM
cell elementwise.

The recurrence is LDWEIGHTS-throughput-bound (64 weight-tile visits per
step at ~32ns each), so the design minimizes PE tile-visits and keeps the
end-of-step serial tail short:
 - i/f/c gate psums get gx added on the Vector engine (off the PE).
 - o-gate gx is injected into PSUM with identity matmuls (hi+lo bf16
   pair) at the START of the step, so the step's tail stays
   sigmoid -> threshold -> h-mult only.
 - phase-1 is split by gate precision: i/f x-projections run as bf16
   matmuls (cheap, accuracy-validated), o/c as f32r. One phase-1 m-tile
   group is interleaved per recurrence step so it executes in the PE gap
   left by the tail.
"""

import sys

for _p in ("/opt/trn_rl_repo",):
    if _p not in sys.path:
        sys.path.append(_p)

import numpy as np
import ml_dtypes

import concourse.bass as bass
import concourse.mybir as mybir
from concourse import tile
from concourse.bass_utils import run_bass_kernel_spmd
from concourse.masks import make_identity

B, T, D, H = 64, 256, 512, 512
NCORES = 8
GROUPS = 4
BS = B // GROUPS          # batch rows per core
NK = H // 128             # contraction tiles over the hidden dim
NM = (4 * H) // 128       # output tiles over the gate dim
BLK_T = 16                # timesteps per phase-1 block
F32 = mybir.dt.float32
F32R = mybir.dt.float32r
BF16 = mybir.dt.bfloat16

# gate column order in W is [i, f, o, c]: m-tiles 0-3=i, 4-7=f, 8-11=o,
# 12-15=c. Process [i, f, c, o] so the o-gate chain is the only
# elementwise work left after the last matmul of a step.
M_ORDER = list(range(0, 8)) + list(range(12, 16)) + list(range(8, 12))


def _patch_tail_drain():
    """This image's walrus rejects more than one sync-wait per engine
    instruction (and any wait on a self-loading 4-byte matmul). Tile
    attaches one wait per outstanding semaphore, so split the excess onto
    nofuse nops committed just before the instruction they guard (same
    engine -> identical semantics)."""
    import bass_rust
    from concourse.vector_clock import ScopedClock

    if getattr(tile.TileContext, "_drain_split_patched", False):
        return

    def _drain_and_barrier(self, tick_clock, wait_clock):
        drain_inst = self.nc.sync.drain()
        wait_clock.add_sem_waits(
            drain_inst.ins, ScopedClock({None: tick_clock.global_clock})
        )
        si = drain_inst.ins.sync_info
        if si is not None and len(si.on_wait) > 1:
            waits = list(si.on_wait)
            drain_inst.ins.sync_info = bass_rust.SyncInfo(
                on_wait=waits[:1], on_update=list(si.on_update)
            )
            for i in range(1, len(waits)):
                nop = self.nc.sync.nop(nofuse=True)
                nop.ins.sync_info = bass_rust.SyncInfo(
                    on_wait=waits[i : i + 1], on_update=[]
                )

        self.nc.all_engine_barrier()
        assert self.sems is not None
        popped = self.nc._tile_sem_poison_stack.pop()
        assert popped is self._sem_poison
        self.nc.clear_and_free_semaphores(list(self.sems.allocated().values()))
        self.nc.all_engine_barrier()

    tile.TileContext._drain_and_barrier = _drain_and_barrier

    orig_commit = tile.TileContext._commit_instruction

    def _commit_instruction(self, inst, lazy_reg_writes: bool = True):
        si = getattr(inst, "sync_info", None)
        limit = 0 if isinstance(inst, mybir.InstMatmult) else 1
        if (
            si is not None
            and len(si.on_wait) > limit
            and inst.engine != mybir.EngineType.Unassigned
        ):
            waits = list(si.on_wait)
            keep = waits[len(waits) - limit :] if limit else []
            for w in waits[: len(waits) - limit]:
                nop = mybir.InstNoOp(
                    name=f"I-{self.nc.next_id()}",
                    sync_info=mybir.SyncInfo(on_wait=[w], on_update=[]),
                    bass_nofuse=True,
                    engine=inst.engine,
                )
                orig_commit(self, nop, lazy_reg_writes=False)
            inst.sync_info = mybir.SyncInfo(
                on_wait=keep, on_update=list(si.on_update)
            )
        return orig_commit(self, inst, lazy_reg_writes)

    tile.TileContext._commit_instruction = _commit_instruction
    tile.TileContext._drain_split_patched = True


def build(with_mask: bool = False, t_steps: int = T):
    """Emit the per-core SPMD module."""
    _patch_tail_drain()
    blk_t = min(BLK_T, t_steps)
    nblk = t_steps // blk_t
    # phase-1 interleave: group (b, j) runs in the tail gap of step
    # 16*(b-1) + j - P1_SHIFT; earlier slots run in the prologue. The
    # shift gives the staged gx stores several steps of slack before the
    # consuming load issues (gxpool prefetches 4 steps ahead and the
    # strided stores take >1us of DMA time each).
    P1_SHIFT = 12

    nc = bass.Bass("TRN2", target_bir_lowering=False, debug=False,
                   num_devices=NCORES)

    # x in both precisions: bf16 feeds the i/f-gate projections, f32r the
    # o/c-gate ones.
    xTb = nc.dram_tensor("xTb", [D, t_steps * BS], BF16, kind="ExternalInput")
    xTf = nc.dram_tensor("xTf", [D, t_steps * BS], F32R, kind="ExternalInput")
    wxb = nc.dram_tensor("wxb", [D, 2 * H], BF16, kind="ExternalInput")
    wxf = nc.dram_tensor("wxf", [D, 2 * H], F32R, kind="ExternalInput")
    wh = nc.dram_tensor("wh", [H, 4 * H], BF16, kind="ExternalInput")
    bt = nc.dram_tensor("bt", [128, NM], F32, kind="ExternalInput")
    msk = (
        nc.dram_tensor("msk", [t_steps, 128, NK * BS], F32,
                       kind="ExternalInput")
        if with_mask
        else None
    )
    hout = nc.dram_tensor("hout", [t_steps, 128, NK * BS], BF16,
                          kind="ExternalOutput")
    # gx scratch, all bf16: slices [0:8]=i/f (single), [8:12]=c hi,
    # [12:16]=c lo, [16:20]=o hi, [20:24]=o lo. hi/lo pairs reconstruct
    # ~fp32 precision when both are injected into the psum accumulator.
    NGX = 24
    gxb = nc.dram_tensor("gx_b", [128, t_steps, NGX * BS], BF16,
                         kind="Internal")

    Act = mybir.ActivationFunctionType
    Alu = mybir.AluOpType

    with tile.TileContext(nc) as tc:
        with (
            tc.tile_pool(name="weights", bufs=1) as wpool,
            tc.tile_pool(name="state", bufs=1) as spool,
            tc.tile_pool(name="p1x", bufs=2) as xpool,
            tc.tile_pool(name="p1stg", bufs=2) as stgpool,
            tc.tile_pool(name="p1psum", bufs=2, space="PSUM") as p1ps,
            tc.tile_pool(name="p2psum", bufs=2, space="PSUM") as p2ps,
            tc.tile_pool(name="p2gx", bufs=8) as gxpool,
            tc.tile_pool(name="p2ew", bufs=2) as ewpool,
        ):
            # weight loads spread across queues so they stream in parallel
            wxbs = wpool.tile([128, NK, 2 * H], BF16)
            nc.gpsimd.dma_start(wxbs[:], wxb.ap().rearrange("(k p) n -> p k n", p=128))
            wxfs = wpool.tile([128, NK, 2 * H], F32R)
            nc.sync.dma_start(wxfs[:], wxf.ap().rearrange("(k p) n -> p k n", p=128))
            whs = wpool.tile([128, NK, 4 * H], BF16)
            nc.scalar.dma_start(whs[:], wh.ap().rearrange("(k p) n -> p k n", p=128))
            bts = wpool.tile([128, NM], F32)
            nc.scalar.dma_start(bts[:], bt.ap())
            ident = wpool.tile([128, 128], BF16)
            make_identity(nc, ident[:])

            # h lives in a ring of 8 slots: step t writes slot t%8 and
            # reads slot (t-1)%8, so the h update never waits on the
            # previous step's readers, and hout is stored in batches of 4
            # steps (one DMA instead of four).
            hring = spool.tile([128, 8, NK, BS], BF16)
            cT = spool.tile([128, NK, BS], F32)
            nc.vector.memset(hring[:], 0.0)
            nc.vector.memset(cT[:], 0.0)

            # ---- phase 1 machinery: gx[t] = x_t @ Wx + b ----
            xvb = xTb.ap().rearrange("(k p) n -> p k n", p=128)
            xvf = xTf.ap().rearrange("(k p) n -> p k n", p=128)
            nfree = blk_t * BS
            p1_tiles: dict = {}
            anchor = [None]       # last recurrence matmul of the step
            anchor_ew = [None]    # last spine elementwise op of the step
            in_prologue = [True]

            def _pin(inst, a, why):
                if a[0] is not None:
                    from concourse.bass import _add_dep_helper
                    _add_dep_helper(inst, a[0], reason=why)

            def p1_group(b, m):
                """One m-tile of one phase-1 block: 4 matmuls + staging +
                per-slice store. Matmuls are pinned behind the current
                step's recurrence matmuls (they fill the tail gap);
                staging ACT/DVE ops are pinned behind the step's spine so
                they never delay it on the engine queues."""
                if b not in p1_tiles:
                    # x loads go on the sync queue so they never sit
                    # behind the slow strided gx stores (gpsimd queue)
                    xb = xpool.tile([128, NK, nfree], BF16, tag="xb")
                    nc.sync.dma_start(
                        xb[:], xvb[:, :, b * nfree : (b + 1) * nfree]
                    )
                    xf = xpool.tile([128, NK, nfree], F32R, tag="xf")
                    nc.sync.dma_start(
                        xf[:], xvf[:, :, b * nfree : (b + 1) * nfree]
                    )
                    stgb = stgpool.tile([128, blk_t, NGX, BS], BF16,
                                        tag="stgb")
                    p1_tiles[b] = (xb, xf, stgb, [0])
                xb, xf, stgb, cnt = p1_tiles[b]
                ps = p1ps.tile([128, nfree], F32, tag="p1ps")
                if m < 8:
                    xsrc, wsrc, col = xb, wxbs, m * 128
                else:
                    xsrc, wsrc, col = xf, wxfs, (m - 8) * 128
                for k in range(NK):
                    mm = nc.tensor.matmul(
                        ps[:],
                        wsrc[:, k, col : col + 128],
                        xsrc[:, k, :],
                        start=(k == 0),
                        stop=(k == NK - 1),
                    )
                    if k == 0:
                        _pin(mm.ins, anchor, "pin p1 group behind its step")
                psv = ps[:].rearrange("p (t b) -> p t b", b=BS)
                dst = gxb.ap().rearrange("p t (m b) -> p t m b", b=BS)
                tslc = slice(b * blk_t, (b + 1) * blk_t)
                if m < 8:
                    hi_slot, lo_slot = m, None       # i/f: single bf16
                elif m >= 12:
                    hi_slot, lo_slot = 8 + (m - 12), 12 + (m - 12)   # c
                else:
                    hi_slot, lo_slot = 16 + (m - 8), 20 + (m - 8)    # o
                # During the prologue (no recurrence traffic yet) spread
                # the slow strided stores over all three DMA queues; in
                # steady state keep them all on gpsimd so they can never
                # delay the gb/x loads (sync) or hout (scalar).
                if in_prologue[0]:
                    qs = (nc.gpsimd, nc.sync, nc.scalar)
                    q0 = qs[(2 * m) % 3]
                    q1 = qs[(2 * m + 1) % 3]
                else:
                    q0 = q1 = nc.gpsimd
                act = nc.scalar.activation(
                    stgb[:, :, hi_slot, :], psv, Act.Identity,
                    bias=bts[:, m : m + 1],
                )
                _pin(act.ins, anchor_ew, "stage after spine")
                q0.dma_start(
                    dst[:, tslc, hi_slot, :], stgb[:, :, hi_slot, :]
                )
                if lo_slot is not None:
                    stt = nc.vector.scalar_tensor_tensor(
                        stgb[:, :, lo_slot, :], psv, bts[:, m : m + 1],
                        stgb[:, :, hi_slot, :], Alu.add, Alu.subtract,
                    )
                    _pin(stt.ins, anchor_ew, "stage after spine")
                    q1.dma_start(
                        dst[:, tslc, lo_slot, :], stgb[:, :, lo_slot, :]
                    )
                cnt[0] += 1
                if cnt[0] == 16:
                    del p1_tiles[b]

            # interleave schedule: one group per step
            sched: dict[int, list] = {}
            prologue: list = []
            for b in range(nblk):
                for j in range(16):
                    s = blk_t * (b - 1) + j - P1_SHIFT
                    if s < 0:
                        prologue.append((b, j))
                    else:
                        sched.setdefault(s, []).append((b, j))
            for b, j in prologue:
                p1_group(b, j)
            in_prologue[0] = False

            # gx injection via identity matmuls (no h dependency). Emitted
            # TWO steps ahead of their step so they execute inside the
            # spine gap of step t-2 (psum buffers free once that step's
            # activations have read them) and stay out of the consuming
            # step's critical prefix.
            live: dict = {}

            def emit_inj(t):
                gb = gxpool.tile([128, NGX, BS], BF16, tag="gb")
                nc.sync.dma_start(
                    gb[:],
                    gxb.ap()[:, t, :].rearrange("p (m b) -> p m b", b=BS),
                )
                ps_if = p2ps.tile([128, 8, BS], F32, tag="ps_if")
                ps_c = p2ps.tile([128, 4, BS], F32, tag="ps_c")
                ps_o = p2ps.tile([128, 4, BS], F32, tag="ps_o")

                def inj(ps_t, sl, start):
                    nc.tensor.matmul(
                        ps_t[:].rearrange("p m b -> p (m b)"), ident[:],
                        gb[:, sl, :].rearrange("p m b -> p (m b)"),
                        start=start, stop=False)
                inj(ps_if, slice(0, 8), True)
                inj(ps_c, slice(8, 12), True)
                inj(ps_c, slice(12, 16), False)
                inj(ps_o, slice(16, 20), True)
                inj(ps_o, slice(20, 24), False)
                live[t] = (gb, ps_if, ps_c, ps_o)

            # ---- phase 2: the recurrence ----
            emit_inj(0)
            if t_steps > 1:
                emit_inj(1)
            for t in range(t_steps):
                hT = hring[:, (t + 7) % 8]  # read state (h of step t-1)
                hN = hring[:, t % 8]        # written state
                gb, ps_if, ps_c, ps_o = live.pop(t)
                for m in M_ORDER:
                    if m < 8:
                        dst = ps_if[:, m, :]
                        last = (m == 7)
                    elif m >= 12:
                        dst = ps_c[:, m - 12, :]
                        last = (m == 15)
                    else:
                        dst = ps_o[:, m - 8, :]
                        last = (m == 11)
                    for k in range(NK):
                        mm = nc.tensor.matmul(
                            dst,
                            whs[:, k, m * 128 : (m + 1) * 128],
                            hT[:, k, :],
                            start=False,
                            stop=last and (k == NK - 1),
                        )
                        anchor[0] = mm.ins
                    if m == 7:
                        sif = ewpool.tile([128, 8, BS], F32, tag="sif")
                        nc.scalar.activation(sif[:], ps_if[:], Act.Sigmoid)
                    elif m == 15:
                        tcg = ewpool.tile([128, NK, BS], F32, tag="tcg")
                        nc.scalar.activation(tcg[:], ps_c[:], Act.Tanh)
                        t1 = ewpool.tile([128, NK, BS], F32, tag="t1")
                        nc.vector.tensor_tensor(
                            t1[:], sif[:, 4:8, :], cT[:], Alu.mult
                        )
                        t2 = ewpool.tile([128, NK, BS], F32, tag="t2")
                        nc.vector.tensor_tensor(
                            t2[:], sif[:, 0:4, :], tcg[:], Alu.mult
                        )
                        if with_mask:
                            cn = ewpool.tile([128, NK, BS], F32, tag="cn")
                            nc.vector.tensor_tensor(cn[:], t1[:], t2[:], Alu.add)
                            mt = ewpool.tile([128, NK * BS], F32, tag="mt")
                            nc.gpsimd.dma_start(mt[:], msk[t])
                            mtv = mt[:].rearrange("p (k b) -> p k b", b=BS)
                            cd = ewpool.tile([128, NK, BS], F32, tag="cd")
                            nc.vector.tensor_tensor(cd[:], cn[:], cT[:],
                                                    Alu.subtract)
                            nc.vector.tensor_tensor(cd[:], cd[:], mtv, Alu.mult)
                            nc.vector.tensor_tensor(cT[:], cT[:], cd[:], Alu.add)
                        else:
                            nc.vector.tensor_tensor(cT[:], t1[:], t2[:], Alu.add)
                        thc = ewpool.tile([128, NK, BS], F32, tag="thc")
                        nc.scalar.activation(thc[:], cT[:], Act.Tanh)
                # output gate chain + h update
                og = ewpool.tile([128, NK, BS], F32, tag="og")
                nc.scalar.activation(og[:], ps_o[:], Act.Sigmoid)
                # threshold(o, 0.4): o if o > 0.4 else 0
                nc.vector.scalar_tensor_tensor(
                    og[:], og[:], 0.4, og[:], Alu.is_gt, Alu.mult
                )
                if with_mask:
                    hn = ewpool.tile([128, NK, BS], F32, tag="hn")
                    nc.vector.tensor_tensor(hn[:], og[:], thc[:], Alu.mult)
                    hd = ewpool.tile([128, NK, BS], F32, tag="hd")
                    nc.vector.tensor_tensor(hd[:], hn[:], hT[:], Alu.subtract)
                    nc.vector.tensor_tensor(hd[:], hd[:], mtv, Alu.mult)
                    hup = nc.vector.tensor_tensor(hN[:], hT[:], hd[:], Alu.add)
                else:
                    hup = nc.vector.tensor_tensor(hN[:], og[:], thc[:],
                                                  Alu.mult)
                anchor_ew[0] = hup.ins
                # hout stored in batches of 4 steps from the h ring (one
                # DMA on the scalar queue instead of four; the ring gives
                # 4 steps of WAR slack before a slot is rewritten).
                if t % 4 == 3 or t == t_steps - 1:
                    t0 = t - t % 4
                    s0 = t0 % 8
                    n = t - t0 + 1
                    nc.scalar.dma_start(
                        hout.ap()[t0 : t + 1].rearrange("t p n -> p t n"),
                        hring[:, s0 : s0 + n].rearrange("p t k b -> p t (k b)"),
                    )

                for b, j in sched.get(t, ()):
                    p1_group(b, j)
                if t + 2 < t_steps:
                    emit_inj(t + 2)
    return nc


_BUILD_CACHE: dict = {}


def _get_module(with_mask: bool, t_steps: int = T):
    key = (with_mask, t_steps)
    if key not in _BUILD_CACHE:
        _BUILD_CACHE[key] = build(with_mask, t_steps)
    return _BUILD_CACHE[key]


def _make_in_maps(x, mask, Wf, bf, Wb, bb, with_mask: bool,
                  t_steps: int = T):
    ws = {}
    for d, (W, bias) in enumerate(((Wf, bf), (Wb, bb))):
        W = np.asarray(W, np.float32)
        wx = np.ascontiguousarray(W[H:])                # [D, 4H] x rows
        ws[d] = (
            np.ascontiguousarray(wx[:, : 2 * H].astype(ml_dtypes.bfloat16)),
            np.ascontiguousarray(wx[:, 2 * H :]),
            np.ascontiguousarray(W[:H].astype(ml_dtypes.bfloat16)),  # wh
            np.ascontiguousarray(
                np.asarray(bias, np.float32).reshape(NM, 128).T
            ),
        )
    in_maps = []
    for core in range(NCORES):
        g, d = core // 2, core % 2
        xs = np.asarray(x[g * BS : (g + 1) * BS, :t_steps], np.float32)
        ms = np.asarray(mask[g * BS : (g + 1) * BS, :t_steps], np.float32)
        if d == 1:
            xs = xs[:, ::-1]
            ms = ms[:, ::-1]
        # xT[dd, t*BS + b] = xs[b, t, dd]
        xTv = np.ascontiguousarray(
            xs.transpose(2, 1, 0).reshape(D, t_steps * BS)
        )
        wxbv, wxfv, whv, btv = ws[d]
        m = {
            "xTb": np.ascontiguousarray(xTv.astype(ml_dtypes.bfloat16)),
            "xTf": xTv,
            "wxb": wxbv,
            "wxf": wxfv,
            "wh": whv,
            "bt": btv,
        }
        if with_mask:
            m["msk"] = np.ascontiguousarray(
                np.broadcast_to(
                    ms.T[:, None, None, :], (t_steps, 128, NK, BS)
                ).reshape(t_steps, 128, NK * BS)
            )
        in_maps.append(m)
    return in_maps


def _assemble(results, t_steps: int = T):
    out = np.empty((B, t_steps, 2 * H), np.float32)
    for core in range(NCORES):
        g, d = core // 2, core % 2
        h = np.asarray(results[core]["hout"], np.float32)  # [t, 128, NK*BS]
        h = h.reshape(t_steps, 128, NK, BS).transpose(3, 0, 2, 1)  # [b,t,k,p]
        h = h.reshape(BS, t_steps, H)
        if d == 1:
            h = h[:, ::-1]
        out[g * BS : (g + 1) * BS, :, d * H : (d + 1) * H] = h
    return out


def run(x, mask, Wf, bf, Wb, bb, trace=False, t_steps: int = T,
        **spmd_kwargs):
    with_mask = not bool(np.all(np.asarray(mask) == 1.0))
    nc = _get_module(with_mask, t_steps)
    in_maps = _make_in_maps(x, mask, Wf, bf, Wb, bb, with_mask, t_steps)
    res = run_bass_kernel_spmd(
        nc, in_maps, list(range(NCORES)), trace=trace, **spmd_kwargs
    )
    return _assemble(res.results, t_steps), res


def kernel(x, mask, Wf, bf, Wb, bb):
    out, _ = run(x, mask, Wf, bf, Wb, bb)
    return out


# revision 37
# speedup vs baseline: 1.3553x; 1.0884x over previous
"""Bi-LSTM Trainium2 kernel: B=64, T=256, D=512, H=512, fp32 I/O.

Sharding: 8 cores = 4 batch groups x 2 directions. Each core runs the full
time recurrence for its 16-sample shard in one direction (the backward
direction is handled by feeding that core a time-reversed input and
un-reversing its output on the host).

On-device layout is fully transposed: hidden/gate dims on SBUF partitions,
batch on the free dim. The recurrent matmul keeps the weight tile as the
stationary (lhsT) operand so the gate output lands transposed in PSUM,
which makes every elementwise op a [128, BS]-shaped op and removes any
per-step transposes.

Phase 1 precomputes gx[t] = x_t @ Wx + b for all t into DRAM scratch;
phase 2 runs the sequential recurrence g = gx[t] + h @ Wh plus the LSTM
cell elementwise.

The recurrence is LDWEIGHTS-throughput-bound (64 weight-tile visits per
step at ~32ns each), so the design minimizes PE tile-visits and keeps the
end-of-step serial tail short:
 - i/f/c gate psums get gx added on the Vector engine (off the PE).
 - o-gate gx is injected into PSUM with identity matmuls (hi+lo bf16
   pair) at the START of the step, so the step's tail stays
   sigmoid -> threshold -> h-mult only.
 - phase-1 is split by gate precision: i/f x-projections run as bf16
   matmuls (cheap, accuracy-validated), o/c as f32r. One phase-1 m-tile
   group is interleaved per recurrence step so it executes in the PE gap
   left by the tail.
"""

import sys

for _p in ("/opt/trn_rl_repo",):
    if _p not in sys.path:
        sys.path.append(_p)

import numpy as np
import ml_dtypes

import concourse.bass as bass
import concourse.mybir as mybir
from concourse import tile
from concourse.bass_utils import run_bass_kernel_spmd
from concourse.masks import make_identity

B, T, D, H = 64, 256, 512, 512
NCORES = 8
GROUPS = 4
BS = B // GROUPS          # batch rows per core
NK = H // 128             # contraction tiles over the hidden dim
NM = (4 * H) // 128       # output tiles over the gate dim
BLK_T = 16                # timesteps per phase-1 block
F32 = mybir.dt.float32
F32R = mybir.dt.float32r
BF16 = mybir.dt.bfloat16

# gate column order in W is [i, f, o, c]: m-tiles 0-3=i, 4-7=f, 8-11=o,
# 12-15=c. Process [i, f, c, o] so the o-gate chain is the only
# elementwise work left after the last matmul of a step.
M_ORDER = list(range(0, 8)) + list(range(12, 16)) + list(range(8, 12))


def _patch_tail_drain():
    """This image's walrus rejects more than one sync-wait per engine
    instruction (and any wait on a self-loading 4-byte matmul). Tile
    attaches one wait per outstanding semaphore, so split the excess onto
    nofuse nops committed just before the instruction they guard (same
    engine -> identical semantics)."""
    import bass_rust
    from concourse.vector_clock import ScopedClock

    if getattr(tile.TileContext, "_drain_split_patched", False):
        return

    def _drain_and_barrier(self, tick_clock, wait_clock):
        drain_inst = self.nc.sync.drain()
        wait_clock.add_sem_waits(
            drain_inst.ins, ScopedClock({None: tick_clock.global_clock})
        )
        si = drain_inst.ins.sync_info
        if si is not None and len(si.on_wait) > 1:
            waits = list(si.on_wait)
            drain_inst.ins.sync_info = bass_rust.SyncInfo(
                on_wait=waits[:1], on_update=list(si.on_update)
            )
            for i in range(1, len(waits)):
                nop = self.nc.sync.nop(nofuse=True)
                nop.ins.sync_info = bass_rust.SyncInfo(
                    on_wait=waits[i : i + 1], on_update=[]
                )

        self.nc.all_engine_barrier()
        assert self.sems is not None
        popped = self.nc._tile_sem_poison_stack.pop()
        assert popped is self._sem_poison
        self.nc.clear_and_free_semaphores(list(self.sems.allocated().values()))
        self.nc.all_engine_barrier()

    tile.TileContext._drain_and_barrier = _drain_and_barrier

    orig_commit = tile.TileContext._commit_instruction

    def _commit_instruction(self, inst, lazy_reg_writes: bool = True):
        si = getattr(inst, "sync_info", None)
        limit = 0 if isinstance(inst, mybir.InstMatmult) else 1
        if (
            si is not None
            and len(si.on_wait) > limit
            and inst.engine != mybir.EngineType.Unassigned
        ):
            waits = list(si.on_wait)
            keep = waits[len(waits) - limit :] if limit else []
            for w in waits[: len(waits) - limit]:
                nop = mybir.InstNoOp(
                    name=f"I-{self.nc.next_id()}",
                    sync_info=mybir.SyncInfo(on_wait=[w], on_update=[]),
                    bass_nofuse=True,
                    engine=inst.engine,
                )
                orig_commit(self, nop, lazy_reg_writes=False)
            inst.sync_info = mybir.SyncInfo(
                on_wait=keep, on_update=list(si.on_update)
            )
        return orig_commit(self, inst, lazy_reg_writes)

    tile.TileContext._commit_instruction = _commit_instruction
    tile.TileContext._drain_split_patched = True


def build(with_mask: bool = False, t_steps: int = T):
    """Emit the per-core SPMD module."""
    _patch_tail_drain()
    blk_t = min(BLK_T, t_steps)
    nblk = t_steps // blk_t
    # phase-1 interleave: group (b, j) runs in the tail gap of step
    # 16*(b-1) + j - P1_SHIFT; earlier slots run in the prologue. The
    # shift gives the staged gx stores several steps of slack before the
    # consuming load issues (gxpool prefetches 4 steps ahead and the
    # strided stores take >1us of DMA time each).
    P1_SHIFT = 12

    nc = bass.Bass("TRN2", target_bir_lowering=False, debug=False,
                   num_devices=NCORES)

    # x in both precisions: bf16 feeds the i/f-gate projections, f32r the
    # o/c-gate ones.
    xTb = nc.dram_tensor("xTb", [D, t_steps * BS], BF16, kind="ExternalInput")
    xTf = nc.dram_tensor("xTf", [D, t_steps * BS], F32R, kind="ExternalInput")
    wxb = nc.dram_tensor("wxb", [D, 2 * H], BF16, kind="ExternalInput")
    wxf = nc.dram_tensor("wxf", [D, 2 * H], F32R, kind="ExternalInput")
    wh = nc.dram_tensor("wh", [H, 4 * H], BF16, kind="ExternalInput")
    bt = nc.dram_tensor("bt", [128, NM], F32, kind="ExternalInput")
    msk = (
        nc.dram_tensor("msk", [t_steps, 128, NK * BS], F32,
                       kind="ExternalInput")
        if with_mask
        else None
    )
    hout = nc.dram_tensor("hout", [t_steps, 128, NK * BS], BF16,
                          kind="ExternalOutput")
    # gx scratch, all bf16: slices [0:8]=i/f (single), [8:12]=c hi,
    # [12:16]=c lo, [16:20]=o hi, [20:24]=o lo. hi/lo pairs reconstruct
    # ~fp32 precision when both are injected into the psum accumulator.
    # Block-major layout [block, slot, t-within-block * batch] makes the
    # per-slot phase-1 stores fully contiguous (the strided access lands
    # on the batched 4-step loads instead, which amortize it).
    NGX = 24
    gxb = nc.dram_tensor("gx_b", [128, nblk, NGX, blk_t * BS], BF16,
                         kind="Internal")

    Act = mybir.ActivationFunctionType
    Alu = mybir.AluOpType

    with tile.TileContext(nc) as tc:
        with (
            tc.tile_pool(name="weights", bufs=1) as wpool,
            tc.tile_pool(name="state", bufs=1) as spool,
            tc.tile_pool(name="p1x", bufs=2) as xpool,
            tc.tile_pool(name="p1stg", bufs=2) as stgpool,
            tc.tile_pool(name="p1psum", bufs=2, space="PSUM") as p1ps,
            tc.tile_pool(name="p2psum", bufs=2, space="PSUM") as p2ps,
            tc.tile_pool(name="p2gx", bufs=4) as gxpool,
            tc.tile_pool(name="p2ew", bufs=2) as ewpool,
        ):
            # weight loads spread across queues so they stream in parallel
            wxbs = wpool.tile([128, NK, 2 * H], BF16)
            nc.gpsimd.dma_start(wxbs[:], wxb.ap().rearrange("(k p) n -> p k n", p=128))
            wxfs = wpool.tile([128, NK, 2 * H], F32R)
            nc.sync.dma_start(wxfs[:], wxf.ap().rearrange("(k p) n -> p k n", p=128))
            whs = wpool.tile([128, NK, 4 * H], BF16)
            nc.scalar.dma_start(whs[:], wh.ap().rearrange("(k p) n -> p k n", p=128))
            bts = wpool.tile([128, NM], F32)
            nc.scalar.dma_start(bts[:], bt.ap())
            ident = wpool.tile([128, 128], BF16)
            make_identity(nc, ident[:])

            # h lives in a ring of 8 slots: step t writes slot t%8 and
            # reads slot (t-1)%8, so the h update never waits on the
            # previous step's readers, and hout is stored in batches of 4
            # steps (one DMA instead of four).
            hring = spool.tile([128, 8, NK, BS], BF16)
            cT = spool.tile([128, NK, BS], F32)
            nc.vector.memset(hring[:], 0.0)
            nc.vector.memset(cT[:], 0.0)

            # ---- phase 1 machinery: gx[t] = x_t @ Wx + b ----
            xvb = xTb.ap().rearrange("(k p) n -> p k n", p=128)
            xvf = xTf.ap().rearrange("(k p) n -> p k n", p=128)
            nfree = blk_t * BS
            p1_tiles: dict = {}
            anchor = [None]       # last recurrence matmul of the step
            anchor_ew = [None]    # last spine elementwise op of the step
            in_prologue = [True]

            def _pin(inst, a, why):
                if a[0] is not None:
                    from concourse.bass import _add_dep_helper
                    _add_dep_helper(inst, a[0], reason=why)

            def p1_group(b, m):
                """One m-tile of one phase-1 block: 4 matmuls + staging +
                per-slice store. Matmuls are pinned behind the current
                step's recurrence matmuls (they fill the tail gap);
                staging ACT/DVE ops are pinned behind the step's spine so
                they never delay it on the engine queues."""
                if b not in p1_tiles:
                    # x loads go on the sync queue so they never sit
                    # behind the slow strided gx stores (gpsimd queue)
                    xb = xpool.tile([128, NK, nfree], BF16, tag="xb")
                    nc.sync.dma_start(
                        xb[:], xvb[:, :, b * nfree : (b + 1) * nfree]
                    )
                    xf = xpool.tile([128, NK, nfree], F32R, tag="xf")
                    nc.sync.dma_start(
                        xf[:], xvf[:, :, b * nfree : (b + 1) * nfree]
                    )
                    stgb = stgpool.tile([128, NGX, blk_t, BS], BF16,
                                        tag="stgb")
                    p1_tiles[b] = (xb, xf, stgb, [0])
                xb, xf, stgb, cnt = p1_tiles[b]
                ps = p1ps.tile([128, nfree], F32, tag="p1ps")
                if m < 8:
                    xsrc, wsrc, col = xb, wxbs, m * 128
                else:
                    xsrc, wsrc, col = xf, wxfs, (m - 8) * 128
                for k in range(NK):
                    mm = nc.tensor.matmul(
                        ps[:],
                        wsrc[:, k, col : col + 128],
                        xsrc[:, k, :],
                        start=(k == 0),
                        stop=(k == NK - 1),
                    )
                    if k == 0:
                        _pin(mm.ins, anchor, "pin p1 group behind its step")
                psv = ps[:].rearrange("p (t b) -> p t b", b=BS)
                if m < 8:
                    hi_slot, lo_slot = m, None       # i/f: single bf16
                elif m >= 12:
                    hi_slot, lo_slot = 8 + (m - 12), 12 + (m - 12)   # c
                else:
                    hi_slot, lo_slot = 16 + (m - 8), 20 + (m - 8)    # o
                # Stores are contiguous per slot; prologue spreads them
                # over all three DMA queues, steady state keeps them on
                # gpsimd away from the gb/x loads (sync) and hout
                # (scalar).
                if in_prologue[0]:
                    qs = (nc.gpsimd, nc.sync, nc.scalar)
                    q0 = qs[(2 * m) % 3]
                    q1 = qs[(2 * m + 1) % 3]
                else:
                    q0 = q1 = nc.gpsimd
                act = nc.scalar.activation(
                    stgb[:, hi_slot], psv, Act.Identity,
                    bias=bts[:, m : m + 1],
                )
                _pin(act.ins, anchor_ew, "stage after spine")
                q0.dma_start(
                    gxb.ap()[:, b, hi_slot],
                    stgb[:, hi_slot].rearrange("p t b -> p (t b)"),
                )
                if lo_slot is not None:
                    stt = nc.vector.scalar_tensor_tensor(
                        stgb[:, lo_slot], psv, bts[:, m : m + 1],
                        stgb[:, hi_slot], Alu.add, Alu.subtract,
                    )
                    _pin(stt.ins, anchor_ew, "stage after spine")
                    q1.dma_start(
                        gxb.ap()[:, b, lo_slot],
                        stgb[:, lo_slot].rearrange("p t b -> p (t b)"),
                    )
                cnt[0] += 1
                if cnt[0] == 16:
                    del p1_tiles[b]

            # interleave schedule: one group per step
            sched: dict[int, list] = {}
            prologue: list = []
            for b in range(nblk):
                for j in range(16):
                    s = blk_t * (b - 1) + j - P1_SHIFT
                    if s < 0:
                        prologue.append((b, j))
                    else:
                        sched.setdefault(s, []).append((b, j))
            for b, j in prologue:
                p1_group(b, j)
            in_prologue[0] = False

            # gx loads are batched 4 steps at a time (one strided DMA per
            # quarter). Injection matmuls are emitted TWO steps ahead of
            # their step so they execute inside the spine gap of step t-2
            # (psum buffers free once that step's activations have read
    	    # them) and stay out of the consuming step's critical prefix.
            qtiles: dict = {}
            live: dict = {}
            QSPAN = 4 if t_steps % 4 == 0 else 1
            nquart = t_steps // QSPAN

            def emit_qload(q):
                gq = gxpool.tile([128, NGX, QSPAN * BS], BF16, tag="gq")
                b, qq = (q * QSPAN) // blk_t, (q * QSPAN) % blk_t
                nc.sync.dma_start(
                    gq[:],
                    gxb.ap()[:, b, :, qq * BS : (qq + QSPAN) * BS],
                )
                qtiles[q] = gq

            def emit_inj(t):
                q, w = t // QSPAN, t % QSPAN
                gq = qtiles[q]
                gb = gq[:, :, w * BS : (w + 1) * BS]
                ps_if = p2ps.tile([128, 8, BS], F32, tag="ps_if")
                ps_c = p2ps.tile([128, 4, BS], F32, tag="ps_c")
                ps_o = p2ps.tile([128, 4, BS], F32, tag="ps_o")

                def inj(ps_t, sl, start):
                    nc.tensor.matmul(
                        ps_t[:], ident[:], gb[:, sl, :],
                        start=start, stop=False)
                inj(ps_if, slice(0, 8), True)
                inj(ps_c, slice(8, 12), True)
                inj(ps_c, slice(12, 16), False)
                inj(ps_o, slice(16, 20), True)
                inj(ps_o, slice(20, 24), False)
                live[t] = (gb, ps_if, ps_c, ps_o)

            # ---- phase 2: the recurrence ----
            emit_qload(0)
            if nquart > 1:
                emit_qload(1)
            emit_inj(0)
            if t_steps > 1:
                emit_inj(1)
            for t in range(t_steps):
                if t % QSPAN == 0 and t // QSPAN + 2 < nquart:
                    emit_qload(t // QSPAN + 2)
                hT = hring[:, (t + 7) % 8]  # read state (h of step t-1)
                hN = hring[:, t % 8]        # written state
                gb, ps_if, ps_c, ps_o = live.pop(t)
                for m in M_ORDER:
                    if m < 8:
                        dst = ps_if[:, m, :]
                        last = (m == 7)
                    elif m >= 12:
                        dst = ps_c[:, m - 12, :]
                        last = (m == 15)
                    else:
                        dst = ps_o[:, m - 8, :]
                        last = (m == 11)
                    for k in range(NK):
                        mm = nc.tensor.matmul(
                            dst,
                            whs[:, k, m * 128 : (m + 1) * 128],
                            hT[:, k, :],
                            start=False,
                            stop=last and (k == NK - 1),
                        )
                        anchor[0] = mm.ins
                    if m == 7:
                        sif = ewpool.tile([128, 8, BS], F32, tag="sif")
                        nc.scalar.activation(sif[:], ps_if[:], Act.Sigmoid)
                    elif m == 15:
                        tcg = ewpool.tile([128, NK, BS], F32, tag="tcg")
                        nc.scalar.activation(tcg[:], ps_c[:], Act.Tanh)
                        t1 = ewpool.tile([128, NK, BS], F32, tag="t1")
                        nc.vector.tensor_tensor(
                            t1[:], sif[:, 4:8, :], cT[:], Alu.mult
                        )
                        t2 = ewpool.tile([128, NK, BS], F32, tag="t2")
                        nc.vector.tensor_tensor(
                            t2[:], sif[:, 0:4, :], tcg[:], Alu.mult
                        )
                        if with_mask:
                            cn = ewpool.tile([128, NK, BS], F32, tag="cn")
                            nc.vector.tensor_tensor(cn[:], t1[:], t2[:], Alu.add)
                            mt = ewpool.tile([128, NK * BS], F32, tag="mt")
                            nc.gpsimd.dma_start(mt[:], msk[t])
                            mtv = mt[:].rearrange("p (k b) -> p k b", b=BS)
                            cd = ewpool.tile([128, NK, BS], F32, tag="cd")
                            nc.vector.tensor_tensor(cd[:], cn[:], cT[:],
                                                    Alu.subtract)
                            nc.vector.tensor_tensor(cd[:], cd[:], mtv, Alu.mult)
                            nc.vector.tensor_tensor(cT[:], cT[:], cd[:], Alu.add)
                        else:
                            nc.vector.tensor_tensor(cT[:], t1[:], t2[:], Alu.add)
                        thc = ewpool.tile([128, NK, BS], F32, tag="thc")
                        nc.scalar.activation(thc[:], cT[:], Act.Tanh)
                # output gate chain + h update
                og = ewpool.tile([128, NK, BS], F32, tag="og")
                nc.scalar.activation(og[:], ps_o[:], Act.Sigmoid)
                # threshold(o, 0.4): o if o > 0.4 else 0
                nc.vector.scalar_tensor_tensor(
                    og[:], og[:], 0.4, og[:], Alu.is_gt, Alu.mult
                )
                if with_mask:
                    hn = ewpool.tile([128, NK, BS], F32, tag="hn")
                    nc.vector.tensor_tensor(hn[:], og[:], thc[:], Alu.mult)
                    hd = ewpool.tile([128, NK, BS], F32, tag="hd")
                    nc.vector.tensor_tensor(hd[:], hn[:], hT[:], Alu.subtract)
                    nc.vector.tensor_tensor(hd[:], hd[:], mtv, Alu.mult)
                    hup = nc.vector.tensor_tensor(hN[:], hT[:], hd[:], Alu.add)
                else:
                    hup = nc.vector.tensor_tensor(hN[:], og[:], thc[:],
                                                  Alu.mult)
                anchor_ew[0] = hup.ins
                # hout stored in batches of 4 steps from the h ring (one
                # DMA on the scalar queue instead of four; the ring gives
                # 4 steps of WAR slack before a slot is rewritten).
                if t % 4 == 3 or t == t_steps - 1:
                    t0 = t - t % 4
                    s0 = t0 % 8
                    n = t - t0 + 1
                    nc.scalar.dma_start(
                        hout.ap()[t0 : t + 1].rearrange("t p n -> p t n"),
                        hring[:, s0 : s0 + n].rearrange("p t k b -> p t (k b)"),
                    )

                for b, j in sched.get(t, ()):
                    p1_group(b, j)
                if t + 2 < t_steps:
                    emit_inj(t + 2)
    return nc


_BUILD_CACHE: dict = {}


def _get_module(with_mask: bool, t_steps: int = T):
    key = (with_mask, t_steps)
    if key not in _BUILD_CACHE:
        _BUILD_CACHE[key] = build(with_mask, t_steps)
    return _BUILD_CACHE[key]


def _make_in_maps(x, mask, Wf, bf, Wb, bb, with_mask: bool,
                  t_steps: int = T):
    ws = {}
    for d, (W, bias) in enumerate(((Wf, bf), (Wb, bb))):
        W = np.asarray(W, np.float32)
        wx = np.ascontiguousarray(W[H:])                # [D, 4H] x rows
        ws[d] = (
            np.ascontiguousarray(wx[:, : 2 * H].astype(ml_dtypes.bfloat16)),
            np.ascontiguousarray(wx[:, 2 * H :]),
            np.ascontiguousarray(W[:H].astype(ml_dtypes.bfloat16)),  # wh
            np.ascontiguousarray(
                np.asarray(bias, np.float32).reshape(NM, 128).T
            ),
        )
    in_maps = []
    for core in range(NCORES):
        g, d = core // 2, core % 2
        xs = np.asarray(x[g * BS : (g + 1) * BS, :t_steps], np.float32)
        ms = np.asarray(mask[g * BS : (g + 1) * BS, :t_steps], np.float32)
        if d == 1:
            xs = xs[:, ::-1]
            ms = ms[:, ::-1]
        # xT[dd, t*BS + b] = xs[b, t, dd]
        xTv = np.ascontiguousarray(
            xs.transpose(2, 1, 0).reshape(D, t_steps * BS)
        )
        wxbv, wxfv, whv, btv = ws[d]
        m = {
            "xTb": np.ascontiguousarray(xTv.astype(ml_dtypes.bfloat16)),
            "xTf": xTv,
            "wxb": wxbv,
            "wxf": wxfv,
            "wh": whv,
            "bt": btv,
        }
        if with_mask:
            m["msk"] = np.ascontiguousarray(
                np.broadcast_to(
                    ms.T[:, None, None, :], (t_steps, 128, NK, BS)
                ).reshape(t_steps, 128, NK * BS)
            )
        in_maps.append(m)
    return in_maps


def _assemble(results, t_steps: int = T):
    out = np.empty((B, t_steps, 2 * H), np.float32)
    for core in range(NCORES):
        g, d = core // 2, core % 2
        h = np.asarray(results[core]["hout"], np.float32)  # [t, 128, NK*BS]
        h = h.reshape(t_steps, 128, NK, BS).transpose(3, 0, 2, 1)  # [b,t,k,p]
        h = h.reshape(BS, t_steps, H)
        if d == 1:
            h = h[:, ::-1]
        out[g * BS : (g + 1) * BS, :, d * H : (d + 1) * H] = h
    return out


def run(x, mask, Wf, bf, Wb, bb, trace=False, t_steps: int = T,
        **spmd_kwargs):
    with_mask = not bool(np.all(np.asarray(mask) == 1.0))
    nc = _get_module(with_mask, t_steps)
    in_maps = _make_in_maps(x, mask, Wf, bf, Wb, bb, with_mask, t_steps)
    res = run_bass_kernel_spmd(
        nc, in_maps, list(range(NCORES)), trace=trace, **spmd_kwargs
    )
    return _assemble(res.results, t_steps), res


def kernel(x, mask, Wf, bf, Wb, bb):
    out, _ = run(x, mask, Wf, bf, Wb, bb)
    return out
